# revision 14
# baseline (speedup 1.0000x reference)
"""Trainium2 Bass kernel for nn_DiagGaussian (MoE routing).

Reference computation:
    action_mean[i] = x[i] @ W[index[i]] + b[index[i]]    # [B, O]
    action_std        = exp(logstd) broadcast to [B, O]

Sharding strategy (expert-parallel): core e owns expert e. The host
groups the batch rows by expert (argsort of index), pads each group to a
fixed capacity C, and ships core e the gathered rows (pre-transposed to
[I, C] so the contraction dim lands on SBUF partitions) plus only its
own W[e]/b[e]. Each core then runs one static dense GEMM - no on-device
routing at all - and the host scatters rows back to their original
positions. Padded output columns are discarded on the host, so pad
lanes are never zeroed.

The GEMM streams x in k-chunks so the PE trails the DMA stream by one
chunk group; DMA issue is spread across the Sync/Scalar/Vector
sequencers (a single sequencer issues DMA_DIRECT2D at only ~1.3/us);
outputs are written per chunk group as one large DMA. A short dummy-
matmul warmup keeps the PE HAM clock-gate at 2.4 GHz when real work
arrives.
"""

import math
import os
import sys
import types

import numpy as np


def _ensure_ntff_hook():
    """Make `antenv.axon_hooks` importable so run_bass_kernel_spmd(trace=True)
    can capture NTFF profiles under axon. The boot shim registers the hook only
    when the image ships an `antenv` package; this container doesn't, so we
    provide the two accessors ourselves, backed by the same ctypes hook."""
    try:
        import antenv.axon_hooks  # noqa: F401
        return
    except ImportError:
        pass
    hook = None
    try:
        from trn_agent_boot.trn_boot import _ntff_profile_via_ctypes
        hook = _ntff_profile_via_ctypes("/opt/axon/libaxon_pjrt.so")
    except Exception:
        pass
    pkg = sys.modules.get("antenv") or types.ModuleType("antenv")
    mod = types.ModuleType("antenv.axon_hooks")
    state = {"hook": hook}
    mod.get_axon_ntff_profile_hook = lambda: state["hook"]
    mod.set_axon_ntff_profile_hook = lambda h: state.__setitem__("hook", h)
    pkg.axon_hooks = mod
    sys.modules.setdefault("antenv", pkg)
    sys.modules["antenv.axon_hooks"] = mod


_ensure_ntff_hook()

import ml_dtypes
import concourse.mybir as mybir
import concourse.tile as tile
from concourse import bacc
from concourse.bass_utils import run_bass_kernel_spmd

B, I, O, E = 16384, 1024, 512, 8
NCORES = 8
KC = I // 128            # 8 contraction chunks of 128
CHUNK_RT = 6             # row-tiles (of 128 rows) per x DMA chunk group
C_DEFAULT = 2176         # per-expert row capacity (mean 2048 + 3 sigma;
                         # bumped at runtime in the unlikely overflow case)
WARMUP_MM = 6           # dummy matmuls to lift the PE HAM clock-gate

DEFAULT_MODE = os.environ.get("KERNEL_DTYPE", "bf16")

_PROGRAM_CACHE = {}
LAST_RESULTS = None      # BassKernelResults of the most recent run (for test harness)


def _build_program(C, mode):
    """Per-core program: y[C,O] = xT.T @ w + bias ; std = exp(logstd)."""
    assert C % 128 == 0
    RT = C // 128                         # row tiles
    groups = []                           # [(first_tile, n_tiles), ...]
    t = 0
    while t < RT:
        n = min(CHUNK_RT, RT - t)
        groups.append((t, n))
        t += n

    if mode == "bf16":
        xw_dt = mybir.dt.bfloat16
    elif mode == "f32r":
        xw_dt = mybir.dt.float32r
    else:
        raise ValueError(mode)

    nc = bacc.Bacc("TRN2", target_bir_lowering=False, debug=False, num_devices=NCORES)
    xT_d = nc.dram_tensor("xT", [I, C], xw_dt, kind="ExternalInput").ap()
    w_d = nc.dram_tensor("w", [I, O], xw_dt, kind="ExternalInput").ap()
    bias_d = nc.dram_tensor("bias", [128, O], mybir.dt.float32, kind="ExternalInput").ap()
    ls_d = nc.dram_tensor("ls", [1, O], mybir.dt.float32, kind="ExternalInput").ap()
    y_d = nc.dram_tensor("y", [C, O], mybir.dt.float32, kind="ExternalOutput").ap()
    std_d = nc.dram_tensor("std", [1, O], mybir.dt.float32, kind="ExternalOutput").ap()

    with tile.TileContext(nc) as tc:
        with (
            tc.tile_pool(name="weights", bufs=1) as wp,
            tc.tile_pool(name="xdata", bufs=KC * len(groups)) as xp,
            tc.tile_pool(name="youts", bufs=4) as op,
            tc.tile_pool(name="ps", bufs=4, space="PSUM") as ps,
            tc.tile_pool(name="pswarm", bufs=1, space="PSUM") as psw,
        ):
            # PE warmup: dummy matmuls (on whatever the SBUF holds - the
            # results are never read) keep the PE busy during the input
            # DMA lag so HAM reaches 2.4 GHz before real matmuls issue.
            warm_sb = wp.tile([128, O], mybir.dt.bfloat16)
            nc.gpsimd.memset(warm_sb[:], 0.0)
            warm_ps = psw.tile([128, O], mybir.dt.float32)
            for _ in range(WARMUP_MM):
                nc.tensor.matmul(
                    warm_ps[:], lhsT=warm_sb[:, :128], rhs=warm_sb[:],
                    start=True, stop=True,
                )

            # Resident weights: all 8 K-chunks side by side [128, 8*512].
            # Split across the two HWDGE queues so the first chunk group
            # isn't serialized behind a single 1 MB transfer.
            w_sb = wp.tile([128, KC * O], xw_dt)
            half = KC // 2
            nc.sync.dma_start(
                out=w_sb[:, :half * O].rearrange("p (k o) -> p k o", k=half),
                in_=w_d[:half * 128].rearrange("(k p) o -> p k o", p=128),
            )
            nc.scalar.dma_start(
                out=w_sb[:, half * O:].rearrange("p (k o) -> p k o", k=KC - half),
                in_=w_d[half * 128:].rearrange("(k p) o -> p k o", p=128),
            )

            # bias/logstd are tiny; gpsimd's SWDGE is idle at the start, so
            # they arrive long before the first psum evacuation needs them.
            bias_sb = wp.tile([128, O], mybir.dt.float32)
            nc.gpsimd.dma_start(out=bias_sb[:], in_=bias_d[:])
            ls_sb = wp.tile([1, O], mybir.dt.float32)
            nc.gpsimd.dma_start(out=ls_sb[:], in_=ls_d[:])

            # x chunks, issued group-major. The first group rides three
            # sequencers (sync/scalar/gpsimd) to minimize the PE lead-in;
            # later groups alternate the two HWDGE queues (gpsimd then
            # handles output tiles) - the steady-state stream outruns the
            # PE 2x either way.
            xch = [[None] * len(groups) for _ in range(KC)]
            for g, (t0, nt) in enumerate(groups):
                for k in range(KC):
                    t = xp.tile([128, CHUNK_RT * 128], xw_dt,
                                name=f"xch_{k}_{g}", tag="xch")
                    if g == 0:
                        eng = (nc.sync, nc.scalar, nc.gpsimd)[k % 3]
                    else:
                        eng = nc.sync if k % 2 == 0 else nc.scalar
                    eng.dma_start(
                        out=t[:, :nt * 128],
                        in_=xT_d[k * 128:(k + 1) * 128, t0 * 128:(t0 + nt) * 128],
                    )
                    xch[k][g] = t

            for g, (t0, nt) in enumerate(groups):
                for j in range(nt):
                    r = t0 + j
                    psum = ps.tile([128, O], mybir.dt.float32)
                    for k in range(KC):
                        nc.tensor.matmul(
                            psum[:],
                            lhsT=xch[k][g][:, j * 128:(j + 1) * 128],
                            rhs=w_sb[:, k * O:(k + 1) * O],
                            start=(k == 0),
                            stop=(k == KC - 1),
                        )
                    yout = op.tile([128, O], mybir.dt.float32,
                                   name=f"yout_{r}", tag="yout")
                    nc.vector.tensor_add(yout[:], psum[:], bias_sb[:])
                    eng = nc.gpsimd if r % 2 == 0 else nc.scalar
                    eng.dma_start(
                        out=y_d[r * 128:(r + 1) * 128, :], in_=yout[:]
                    )

            # exp(logstd) — tiny one-shot, emitted last so the ACT table
            # load doesn't sit in front of Scalar's DMA issues.
            std_sb = wp.tile([1, O], mybir.dt.float32)
            nc.scalar.activation(std_sb[:], ls_sb[:], mybir.ActivationFunctionType.Exp)
            nc.gpsimd.dma_start(out=std_d[:], in_=std_sb[:])

    nc.compile()
    return nc


def kernel(x, index, W, b, logstd, *, trace=False, mode=None):
    global LAST_RESULTS
    mode = mode or DEFAULT_MODE
    x = np.ascontiguousarray(x, dtype=np.float32)
    W = np.ascontiguousarray(W, dtype=np.float32)
    b = np.ascontiguousarray(b, dtype=np.float32)
    logstd = np.ascontiguousarray(logstd, dtype=np.float32)
    idx = np.asarray(index).astype(np.int64)

    order = np.argsort(idx, kind="stable")
    counts = np.bincount(idx, minlength=E)
    starts = np.zeros(E + 1, dtype=np.int64)
    np.cumsum(counts, out=starts[1:])

    C = max(C_DEFAULT, math.ceil(counts.max() / 128) * 128)
    key = (C, mode)
    nc = _PROGRAM_CACHE.get(key)
    if nc is None:
        nc = _PROGRAM_CACHE[key] = _build_program(C, mode)

    np_dt = ml_dtypes.bfloat16 if mode == "bf16" else np.float32
    in_maps = []
    for e in range(E):
        rows = order[starts[e]:starts[e + 1]]
        xgT = np.zeros((I, C), dtype=np_dt)
        xgT[:, :len(rows)] = x[rows].astype(np_dt).T
        in_maps.append({
            "xT": xgT,
            "w": W[e].astype(np_dt),
            "bias": np.ascontiguousarray(np.broadcast_to(b[e], (128, O))),
            "ls": logstd.reshape(1, O),
        })

    res = run_bass_kernel_spmd(nc, in_maps, list(range(NCORES)), trace=trace)
    LAST_RESULTS = res

    action_mean = np.empty((B, O), dtype=np.float32)
    for e in range(E):
        n = counts[e]
        action_mean[order[starts[e]:starts[e + 1]]] = res.results[e]["y"][:n]
    std_row = res.results[0]["std"][0]
    action_std = np.ascontiguousarray(np.broadcast_to(std_row, (B, O)))
    return action_mean, action_std


# revision 17
# speedup vs baseline: 1.0045x; 1.0045x over previous
"""Trainium2 Bass kernel for nn_DiagGaussian (MoE routing).

Reference computation:
    action_mean[i] = x[i] @ W[index[i]] + b[index[i]]    # [B, O]
    action_std        = exp(logstd) broadcast to [B, O]

Sharding strategy (expert-parallel): core e owns expert e. The host
groups the batch rows by expert (argsort of index), pads each group to a
fixed capacity C, and ships core e the gathered rows (pre-transposed to
[I, C] so the contraction dim lands on SBUF partitions) plus only its
own W[e]/b[e]. Each core then runs one static dense GEMM - no on-device
routing at all - and the host scatters rows back to their original
positions. Padded output columns are discarded on the host, so pad
lanes are never zeroed.

The GEMM streams x in k-chunks so the PE trails the DMA stream by one
chunk group; DMA issue is spread across the Sync/Scalar/Vector
sequencers (a single sequencer issues DMA_DIRECT2D at only ~1.3/us);
outputs are written per chunk group as one large DMA. A short dummy-
matmul warmup keeps the PE HAM clock-gate at 2.4 GHz when real work
arrives.
"""

import math
import os
import sys
import types

import numpy as np


def _ensure_ntff_hook():
    """Make `antenv.axon_hooks` importable so run_bass_kernel_spmd(trace=True)
    can capture NTFF profiles under axon. The boot shim registers the hook only
    when the image ships an `antenv` package; this container doesn't, so we
    provide the two accessors ourselves, backed by the same ctypes hook."""
    try:
        import antenv.axon_hooks  # noqa: F401
        return
    except ImportError:
        pass
    hook = None
    try:
        from trn_agent_boot.trn_boot import _ntff_profile_via_ctypes
        hook = _ntff_profile_via_ctypes("/opt/axon/libaxon_pjrt.so")
    except Exception:
        pass
    pkg = sys.modules.get("antenv") or types.ModuleType("antenv")
    mod = types.ModuleType("antenv.axon_hooks")
    state = {"hook": hook}
    mod.get_axon_ntff_profile_hook = lambda: state["hook"]
    mod.set_axon_ntff_profile_hook = lambda h: state.__setitem__("hook", h)
    pkg.axon_hooks = mod
    sys.modules.setdefault("antenv", pkg)
    sys.modules["antenv.axon_hooks"] = mod


_ensure_ntff_hook()

import ml_dtypes
import concourse.mybir as mybir
import concourse.tile as tile
from concourse import bacc
from concourse.bass_utils import run_bass_kernel_spmd

B, I, O, E = 16384, 1024, 512, 8
NCORES = 8
KC = I // 128            # 8 contraction chunks of 128
CHUNK_RT = 6             # row-tiles (of 128 rows) per x DMA chunk group
C_DEFAULT = 2176         # per-expert row capacity (mean 2048 + 3 sigma;
                         # bumped at runtime in the unlikely overflow case)
WARMUP_MM = 8           # dummy matmuls to lift the PE HAM clock-gate

DEFAULT_MODE = os.environ.get("KERNEL_DTYPE", "bf16")

_PROGRAM_CACHE = {}
LAST_RESULTS = None      # BassKernelResults of the most recent run (for test harness)


def _build_program(C, mode):
    """Per-core program: y[C,O] = xT.T @ w + bias ; std = exp(logstd)."""
    assert C % 128 == 0
    RT = C // 128                         # row tiles
    groups = []                           # [(first_tile, n_tiles), ...]
    t = 0
    # Ramped group sizes: a small first group lets the PE start right
    # after the warmup instead of waiting for a full 6-tile delivery;
    # the DMA stream outruns the PE ~2x, so later groups can be bigger.
    for n in (2, 4):
        if t + n <= RT:
            groups.append((t, n))
            t += n
    while t < RT:
        n = min(CHUNK_RT, RT - t)
        groups.append((t, n))
        t += n

    if mode == "bf16":
        xw_dt = mybir.dt.bfloat16
    elif mode == "f32r":
        xw_dt = mybir.dt.float32r
    else:
        raise ValueError(mode)

    nc = bacc.Bacc("TRN2", target_bir_lowering=False, debug=False, num_devices=NCORES)
    xT_d = nc.dram_tensor("xT", [I, C], xw_dt, kind="ExternalInput").ap()
    w_d = nc.dram_tensor("w", [I, O], xw_dt, kind="ExternalInput").ap()
    bias_d = nc.dram_tensor("bias", [128, O], mybir.dt.float32, kind="ExternalInput").ap()
    ls_d = nc.dram_tensor("ls", [1, O], mybir.dt.float32, kind="ExternalInput").ap()
    y_d = nc.dram_tensor("y", [C, O], mybir.dt.float32, kind="ExternalOutput").ap()
    std_d = nc.dram_tensor("std", [1, O], mybir.dt.float32, kind="ExternalOutput").ap()

    with tile.TileContext(nc) as tc:
        with (
            tc.tile_pool(name="weights", bufs=1) as wp,
            tc.tile_pool(name="xdata", bufs=KC * len(groups)) as xp,
            tc.tile_pool(name="youts", bufs=4) as op,
            tc.tile_pool(name="ps", bufs=4, space="PSUM") as ps,
            tc.tile_pool(name="pswarm", bufs=1, space="PSUM") as psw,
        ):
            # PE warmup: dummy matmuls (on whatever the SBUF holds - the
            # results are never read) keep the PE busy during the input
            # DMA lag so HAM reaches 2.4 GHz before real matmuls issue.
            warm_sb = wp.tile([128, O], mybir.dt.bfloat16)
            nc.gpsimd.memset(warm_sb[:], 0.0)
            warm_ps = psw.tile([128, O], mybir.dt.float32)
            for _ in range(WARMUP_MM):
                nc.tensor.matmul(
                    warm_ps[:], lhsT=warm_sb[:, :128], rhs=warm_sb[:],
                    start=True, stop=True,
                )

            # Resident weights: all 8 K-chunks side by side [128, 8*512].
            # Split across the two HWDGE queues so the first chunk group
            # isn't serialized behind a single 1 MB transfer.
            w_sb = wp.tile([128, KC * O], xw_dt)
            half = KC // 2
            nc.sync.dma_start(
                out=w_sb[:, :half * O].rearrange("p (k o) -> p k o", k=half),
                in_=w_d[:half * 128].rearrange("(k p) o -> p k o", p=128),
            )
            nc.scalar.dma_start(
                out=w_sb[:, half * O:].rearrange("p (k o) -> p k o", k=KC - half),
                in_=w_d[half * 128:].rearrange("(k p) o -> p k o", p=128),
            )

            # bias/logstd are tiny; gpsimd's SWDGE is idle at the start, so
            # they arrive long before the first psum evacuation needs them.
            bias_sb = wp.tile([128, O], mybir.dt.float32)
            nc.gpsimd.dma_start(out=bias_sb[:], in_=bias_d[:])
            ls_sb = wp.tile([1, O], mybir.dt.float32)
            nc.gpsimd.dma_start(out=ls_sb[:], in_=ls_d[:])

            # x chunks, issued group-major. The first group rides three
            # sequencers (sync/scalar/gpsimd) to minimize the PE lead-in;
            # later groups alternate the two HWDGE queues (gpsimd then
            # handles output tiles) - the steady-state stream outruns the
            # PE 2x either way.
            xch = [[None] * len(groups) for _ in range(KC)]
            for g, (t0, nt) in enumerate(groups):
                for k in range(KC):
                    t = xp.tile([128, CHUNK_RT * 128], xw_dt,
                                name=f"xch_{k}_{g}", tag="xch")
                    eng = nc.sync if k % 2 == 0 else nc.scalar
                    eng.dma_start(
                        out=t[:, :nt * 128],
                        in_=xT_d[k * 128:(k + 1) * 128, t0 * 128:(t0 + nt) * 128],
                    )
                    xch[k][g] = t

            for g, (t0, nt) in enumerate(groups):
                for j in range(nt):
                    r = t0 + j
                    psum = ps.tile([128, O], mybir.dt.float32)
                    for k in range(KC):
                        nc.tensor.matmul(
                            psum[:],
                            lhsT=xch[k][g][:, j * 128:(j + 1) * 128],
                            rhs=w_sb[:, k * O:(k + 1) * O],
                            start=(k == 0),
                            stop=(k == KC - 1),
                        )
                    yout = op.tile([128, O], mybir.dt.float32,
                                   name=f"yout_{r}", tag="yout")
                    nc.vector.tensor_add(yout[:], psum[:], bias_sb[:])
                    eng = nc.gpsimd if r % 2 == 0 else nc.scalar
                    eng.dma_start(
                        out=y_d[r * 128:(r + 1) * 128, :], in_=yout[:]
                    )

            # exp(logstd) — tiny one-shot, emitted last so the ACT table
            # load doesn't sit in front of Scalar's DMA issues.
            std_sb = wp.tile([1, O], mybir.dt.float32)
            nc.scalar.activation(std_sb[:], ls_sb[:], mybir.ActivationFunctionType.Exp)
            nc.gpsimd.dma_start(out=std_d[:], in_=std_sb[:])

    nc.compile()
    return nc


def kernel(x, index, W, b, logstd, *, trace=False, mode=None):
    global LAST_RESULTS
    mode = mode or DEFAULT_MODE
    x = np.ascontiguousarray(x, dtype=np.float32)
    W = np.ascontiguousarray(W, dtype=np.float32)
    b = np.ascontiguousarray(b, dtype=np.float32)
    logstd = np.ascontiguousarray(logstd, dtype=np.float32)
    idx = np.asarray(index).astype(np.int64)

    order = np.argsort(idx, kind="stable")
    counts = np.bincount(idx, minlength=E)
    starts = np.zeros(E + 1, dtype=np.int64)
    np.cumsum(counts, out=starts[1:])

    C = max(C_DEFAULT, math.ceil(counts.max() / 128) * 128)
    key = (C, mode)
    nc = _PROGRAM_CACHE.get(key)
    if nc is None:
        nc = _PROGRAM_CACHE[key] = _build_program(C, mode)

    np_dt = ml_dtypes.bfloat16 if mode == "bf16" else np.float32
    in_maps = []
    for e in range(E):
        rows = order[starts[e]:starts[e + 1]]
        xgT = np.zeros((I, C), dtype=np_dt)
        xgT[:, :len(rows)] = x[rows].astype(np_dt).T
        in_maps.append({
            "xT": xgT,
            "w": W[e].astype(np_dt),
            "bias": np.ascontiguousarray(np.broadcast_to(b[e], (128, O))),
            "ls": logstd.reshape(1, O),
        })

    res = run_bass_kernel_spmd(nc, in_maps, list(range(NCORES)), trace=trace)
    LAST_RESULTS = res

    action_mean = np.empty((B, O), dtype=np.float32)
    for e in range(E):
        n = counts[e]
        action_mean[order[starts[e]:starts[e + 1]]] = res.results[e]["y"][:n]
    std_row = res.results[0]["std"][0]
    action_std = np.ascontiguousarray(np.broadcast_to(std_row, (B, O)))
    return action_mean, action_std


# revision 24
# speedup vs baseline: 1.0475x; 1.0428x over previous
"""Trainium2 Bass kernel for nn_DiagGaussian (MoE routing).

Reference computation:
    action_mean[i] = x[i] @ W[index[i]] + b[index[i]]    # [B, O]
    action_std        = exp(logstd) broadcast to [B, O]

Sharding strategy (expert-parallel): core e owns expert e. The host
groups the batch rows by expert (argsort of index), pads each group to a
fixed capacity C, and ships core e the gathered rows (pre-transposed to
[I, C] so the contraction dim lands on SBUF partitions) plus only its
own W[e]/b[e]. Each core then runs one static dense GEMM - no on-device
routing at all - and the host scatters rows back to their original
positions. Padded output columns are discarded on the host, so pad
lanes are never zeroed.

Inside each core the GEMM is a Tile-scheduled stream: x arrives in
6-row-tile k-chunks split over the Sync and Scalar HWDGE queues (one
sequencer issues DMA_DIRECT2D at only ~1.3/us, so issue is spread);
matmuls (bf16, N=512, fp32 PSUM accumulate) trail the stream by one
chunk group; the DVE fuses bias-add with PSUM evacuation; outputs DMA
per row-tile from gpsimd/scalar. A short dummy-matmul warmup bridges
the input lead-in so the PE HAM clock-gate is at 2.4 GHz when real
work arrives. Modes: bf16 (default, ~2.5e-3 scale-rel err), f32r
(TF32-like, ~1.6e-4, ~35% slower), and *-raw variants that bypass
TileContext with hand-rolled semaphores (no faster in practice).
"""

import math
import os
import sys
import types

import numpy as np


def _ensure_ntff_hook():
    """Make `antenv.axon_hooks` importable so run_bass_kernel_spmd(trace=True)
    can capture NTFF profiles under axon. The boot shim registers the hook only
    when the image ships an `antenv` package; this container doesn't, so we
    provide the two accessors ourselves, backed by the same ctypes hook."""
    try:
        import antenv.axon_hooks  # noqa: F401
        return
    except ImportError:
        pass
    hook = None
    try:
        from trn_agent_boot.trn_boot import _ntff_profile_via_ctypes
        hook = _ntff_profile_via_ctypes("/opt/axon/libaxon_pjrt.so")
    except Exception:
        pass
    pkg = sys.modules.get("antenv") or types.ModuleType("antenv")
    mod = types.ModuleType("antenv.axon_hooks")
    state = {"hook": hook}
    mod.get_axon_ntff_profile_hook = lambda: state["hook"]
    mod.set_axon_ntff_profile_hook = lambda h: state.__setitem__("hook", h)
    pkg.axon_hooks = mod
    sys.modules.setdefault("antenv", pkg)
    sys.modules["antenv.axon_hooks"] = mod


_ensure_ntff_hook()

import ml_dtypes
import concourse.mybir as mybir
import concourse.tile as tile
from concourse import bacc
from concourse.bass_utils import run_bass_kernel_spmd

B, I, O, E = 16384, 1024, 512, 8
NCORES = 8
KC = I // 128            # 8 contraction chunks of 128
CHUNK_RT = 6             # row-tiles (of 128 rows) per x DMA chunk group
C_DEFAULT = 2176         # per-expert row capacity (mean 2048 + 3 sigma;
                         # bumped at runtime in the unlikely overflow case)
WARMUP_MM = 10           # dummy matmuls to lift the PE HAM clock-gate

DEFAULT_MODE = os.environ.get("KERNEL_DTYPE", "bf16")

_PROGRAM_CACHE = {}
LAST_RESULTS = None      # BassKernelResults of the most recent run (for test harness)


F32 = mybir.dt.float32


def _build_program_raw(C, mode):
    RT = C // 128
    groups = []
    t = 0
    while t < RT:
        n = min(CHUNK_RT, RT - t)
        groups.append((t, n))
        t += n

    xw_dt = mybir.dt.bfloat16 if mode.startswith("bf16") else mybir.dt.float32r

    nc = bacc.Bacc("TRN2", target_bir_lowering=False, debug=False, num_devices=NCORES)
    xT_d = nc.dram_tensor("xT", [I, C], xw_dt, kind="ExternalInput").ap()
    w_d = nc.dram_tensor("w", [I, O], xw_dt, kind="ExternalInput").ap()
    bias_d = nc.dram_tensor("bias", [128, O], F32, kind="ExternalInput").ap()
    ls_d = nc.dram_tensor("ls", [1, O], F32, kind="ExternalInput").ap()
    y_d = nc.dram_tensor("y", [C, O], F32, kind="ExternalOutput").ap()
    std_d = nc.dram_tensor("std", [1, O], F32, kind="ExternalOutput").ap()

    # SBUF / PSUM
    w_sb = nc.alloc_sbuf_tensor("w_sb", [128, KC * O], xw_dt).ap()
    bias_sb = nc.alloc_sbuf_tensor("bias_sb", [128, O], F32).ap()
    ls_sb = nc.alloc_sbuf_tensor("ls_sb", [1, O], F32).ap()
    std_sb = nc.alloc_sbuf_tensor("std_sb", [1, O], F32).ap()
    warm_sb = nc.alloc_sbuf_tensor("warm_sb", [128, O], mybir.dt.bfloat16).ap()
    xch = [[None] * len(groups) for _ in range(KC)]
    for g, (t0, nt) in enumerate(groups):
        for k in range(KC):
            xch[k][g] = nc.alloc_sbuf_tensor(
                f"xch_{k}_{g}", [128, nt * 128], xw_dt).ap()
    youts = [nc.alloc_sbuf_tensor(f"yout{i}", [128, O], F32).ap() for i in range(4)]
    psums = [nc.alloc_psum_tensor(f"psum{i}", [128, O], F32).ap() for i in range(4)]
    warm_ps = nc.alloc_psum_tensor("warm_ps", [128, O], F32).ap()

    # Semaphores
    s_w = [nc.alloc_semaphore(f"s_w{i}") for i in range(2)]
    s_ck = [[nc.alloc_semaphore(f"s_ck_{k}_{g}") for g in range(len(groups))]
            for k in range(KC)]
    s_bias = nc.alloc_semaphore("s_bias")
    s_ls = nc.alloc_semaphore("s_ls")
    s_std = nc.alloc_semaphore("s_std")
    pe_sem = nc.alloc_semaphore("pe_sem")
    dve_sem = nc.alloc_semaphore("dve_sem")
    s_yslot = [nc.alloc_semaphore(f"s_yslot{i}") for i in range(4)]
    s_warm = nc.alloc_semaphore("s_warm")
    s_exp = nc.alloc_semaphore("s_exp")
    all_sems = (s_w + [s for row in s_ck for s in row]
                + [s_bias, s_ls, s_std, pe_sem, dve_sem, s_warm, s_exp] + s_yslot)

    half = KC // 2

    # ---- SP stream: first w half + even-k x chunks
    nc.sync.dma_start(
        out=w_sb[:, :half * O].rearrange("p (k o) -> p k o", k=half),
        in_=w_d[:half * 128].rearrange("(k p) o -> p k o", p=128),
    ).then_inc(s_w[0], 16)
    for g, (t0, nt) in enumerate(groups):
        for k in range(0, KC, 2):
            nc.sync.dma_start(
                out=xch[k][g][:],
                in_=xT_d[k * 128:(k + 1) * 128, t0 * 128:(t0 + nt) * 128],
            ).then_inc(s_ck[k][g], 16)

    # ---- ACT stream: second w half + odd-k x chunks, then odd-r outputs, exp
    nc.scalar.dma_start(
        out=w_sb[:, half * O:].rearrange("p (k o) -> p k o", k=KC - half),
        in_=w_d[half * 128:].rearrange("(k p) o -> p k o", p=128),
    ).then_inc(s_w[1], 16)
    for g, (t0, nt) in enumerate(groups):
        for k in range(1, KC, 2):
            nc.scalar.dma_start(
                out=xch[k][g][:],
                in_=xT_d[k * 128:(k + 1) * 128, t0 * 128:(t0 + nt) * 128],
            ).then_inc(s_ck[k][g], 16)
    for r in range(1, RT, 2):
        nc.scalar.wait_ge(dve_sem, r + 1)
        nc.scalar.dma_start(
            out=y_d[r * 128:(r + 1) * 128, :], in_=youts[r % 4][:]
        ).then_inc(s_yslot[r % 4], 16)
    nc.scalar.wait_ge(s_ls, 16)
    nc.scalar.activation(std_sb[:], ls_sb[:], mybir.ActivationFunctionType.Exp
                         ).then_inc(s_exp, 1)
    nc.scalar.wait_ge(s_exp, 1)
    nc.scalar.dma_start(out=std_d[:], in_=std_sb[:]).then_inc(s_std, 16)

    # ---- POOL stream: warmup-tile memset, bias/ls, even-r outputs,
    # final waits + sem reset
    nc.gpsimd.memset(warm_sb[:], 0.0).then_inc(s_warm, 1)
    nc.gpsimd.dma_start(out=bias_sb[:], in_=bias_d[:]).then_inc(s_bias, 16)
    nc.gpsimd.dma_start(out=ls_sb[:], in_=ls_d[:]).then_inc(s_ls, 16)
    for r in range(0, RT, 2):
        nc.gpsimd.wait_ge(dve_sem, r + 1)
        nc.gpsimd.dma_start(
            out=y_d[r * 128:(r + 1) * 128, :], in_=youts[r % 4][:]
        ).then_inc(s_yslot[r % 4], 16)
    for b in range(4):
        total = len(range(b, RT, 4))
        nc.gpsimd.wait_ge(s_yslot[b], 16 * total)
    nc.gpsimd.wait_ge(s_std, 16)
    # Observe every semaphore's final value before the range-clear: the
    # clear must happen-after all updates, and transitive sync through
    # other engines isn't accepted by the checker.
    nc.gpsimd.wait_ge(s_w[0], 16)
    nc.gpsimd.wait_ge(s_w[1], 16)
    for k in range(KC):
        for g in range(len(groups)):
            nc.gpsimd.wait_ge(s_ck[k][g], 16)
    nc.gpsimd.wait_ge(s_bias, 16)
    nc.gpsimd.wait_ge(s_ls, 16)
    nc.gpsimd.wait_ge(s_warm, 1)
    nc.gpsimd.wait_ge(s_exp, 1)
    nc.gpsimd.wait_ge(pe_sem, RT)
    nc.gpsimd.wait_ge(dve_sem, RT)

    # ---- PE stream: warmup + the GEMM
    nc.tensor.wait_ge(s_warm, 1)
    for _ in range(WARMUP_MM):
        nc.tensor.matmul(warm_ps[:], lhsT=warm_sb[:, :128], rhs=warm_sb[:],
                         start=True, stop=True)
    for g, (t0, nt) in enumerate(groups):
        for j in range(nt):
            r = t0 + j
            if j == 0:
                if g == 0:
                    nc.tensor.wait_ge(s_w[0], 16)
                    nc.tensor.wait_ge(s_w[1], 16)
                for k in range(KC):
                    nc.tensor.wait_ge(s_ck[k][g], 16)
            if r >= 4:
                nc.tensor.wait_ge(dve_sem, r - 3)
            for k in range(KC):
                inst = nc.tensor.matmul(
                    psums[r % 4][:],
                    lhsT=xch[k][g][:, j * 128:(j + 1) * 128],
                    rhs=w_sb[:, k * O:(k + 1) * O],
                    start=(k == 0),
                    stop=(k == KC - 1),
                )
            inst.then_inc(pe_sem, 1)

    # ---- DVE stream: psum evacuation + bias
    nc.vector.wait_ge(s_bias, 16)
    for r in range(RT):
        nc.vector.wait_ge(pe_sem, r + 1)
        if r >= 4:
            nc.vector.wait_ge(s_yslot[r % 4], 16 * (r // 4))
        nc.vector.tensor_add(youts[r % 4][:], psums[r % 4][:], bias_sb[:]
                             ).then_inc(dve_sem, 1)

    # Emitted last so these land at the END of every engine's stream:
    # a cheap sem-only barrier (no engine/DGE drains) propagates POOL's
    # fully-synced view to all engines, then POOL range-clears the
    # semaphores so a re-execution of the NEFF starts from zero.
    nc.all_engine_barrier(sem_only=True)
    nc.clear_and_free_semaphores(all_sems)

    nc.compile()
    return nc


def _build_program(C, mode):
    """Per-core program: y[C,O] = xT.T @ w + bias ; std = exp(logstd)."""
    assert C % 128 == 0
    RT = C // 128                         # row tiles
    groups = []                           # [(first_tile, n_tiles), ...]
    t = 0
    while t < RT:
        n = min(CHUNK_RT, RT - t)
        groups.append((t, n))
        t += n

    if mode.startswith("bf16"):
        xw_dt = mybir.dt.bfloat16
    elif mode.startswith("f32r"):
        xw_dt = mybir.dt.float32r
    else:
        raise ValueError(mode)

    nc = bacc.Bacc("TRN2", target_bir_lowering=False, debug=False, num_devices=NCORES)
    xT_d = nc.dram_tensor("xT", [I, C], xw_dt, kind="ExternalInput").ap()
    w_d = nc.dram_tensor("w", [I, O], xw_dt, kind="ExternalInput").ap()
    bias_d = nc.dram_tensor("bias", [128, O], mybir.dt.float32, kind="ExternalInput").ap()
    ls_d = nc.dram_tensor("ls", [1, O], mybir.dt.float32, kind="ExternalInput").ap()
    y_d = nc.dram_tensor("y", [C, O], mybir.dt.float32, kind="ExternalOutput").ap()
    std_d = nc.dram_tensor("std", [1, O], mybir.dt.float32, kind="ExternalOutput").ap()

    with tile.TileContext(nc) as tc:
        with (
            tc.tile_pool(name="weights", bufs=1) as wp,
            tc.tile_pool(name="xdata", bufs=KC * len(groups)) as xp,
            tc.tile_pool(name="youts", bufs=6) as op,
            tc.tile_pool(name="ps", bufs=6, space="PSUM") as ps,
            tc.tile_pool(name="pswarm", bufs=1, space="PSUM") as psw,
        ):
            # PE warmup: dummy matmuls (on whatever the SBUF holds - the
            # results are never read) keep the PE busy during the input
            # DMA lag so HAM reaches 2.4 GHz before real matmuls issue.
            warm_sb = wp.tile([128, O], mybir.dt.bfloat16)
            nc.gpsimd.memset(warm_sb[:], 0.0)
            warm_ps = psw.tile([128, O], mybir.dt.float32)
            for _ in range(WARMUP_MM):
                nc.tensor.matmul(
                    warm_ps[:], lhsT=warm_sb[:, :128], rhs=warm_sb[:],
                    start=True, stop=True,
                )

            # Resident weights: all 8 K-chunks side by side [128, 8*512].
            # Split across the two HWDGE queues so the first chunk group
            # isn't serialized behind a single 1 MB transfer.
            w_sb = wp.tile([128, KC * O], xw_dt)
            half = KC // 2
            nc.sync.dma_start(
                out=w_sb[:, :half * O].rearrange("p (k o) -> p k o", k=half),
                in_=w_d[:half * 128].rearrange("(k p) o -> p k o", p=128),
            )
            nc.scalar.dma_start(
                out=w_sb[:, half * O:].rearrange("p (k o) -> p k o", k=KC - half),
                in_=w_d[half * 128:].rearrange("(k p) o -> p k o", p=128),
            )
            bias_sb = wp.tile([128, O], mybir.dt.float32)
            nc.gpsimd.dma_start(out=bias_sb[:], in_=bias_d[:])
            ls_sb = wp.tile([1, O], mybir.dt.float32)
            nc.gpsimd.dma_start(out=ls_sb[:], in_=ls_d[:])

            # x chunks, issued group-major. The first group rides three
            # sequencers (sync/scalar/gpsimd) to minimize the PE lead-in;
            # later groups alternate the two HWDGE queues (gpsimd then
            # handles output tiles) - the steady-state stream outruns the
            # PE 2x either way.
            xch = [[None] * len(groups) for _ in range(KC)]
            for g, (t0, nt) in enumerate(groups):
                for k in range(KC):
                    t = xp.tile([128, CHUNK_RT * 128], xw_dt,
                                name=f"xch_{k}_{g}", tag="xch")
                    eng = nc.sync if k % 2 == 0 else nc.scalar
                    eng.dma_start(
                        out=t[:, :nt * 128],
                        in_=xT_d[k * 128:(k + 1) * 128, t0 * 128:(t0 + nt) * 128],
                    )
                    xch[k][g] = t

            for g, (t0, nt) in enumerate(groups):
                for j in range(nt):
                    r = t0 + j
                    psum = ps.tile([128, O], mybir.dt.float32)
                    for k in range(KC):
                        nc.tensor.matmul(
                            psum[:],
                            lhsT=xch[k][g][:, j * 128:(j + 1) * 128],
                            rhs=w_sb[:, k * O:(k + 1) * O],
                            start=(k == 0),
                            stop=(k == KC - 1),
                        )
                    yout = op.tile([128, O], mybir.dt.float32,
                                   name=f"yout_{r}", tag="yout")
                    nc.vector.tensor_add(yout[:], psum[:], bias_sb[:])
                    eng = nc.gpsimd if r % 2 == 0 else nc.scalar
                    eng.dma_start(
                        out=y_d[r * 128:(r + 1) * 128, :], in_=yout[:]
                    )

            # exp(logstd) — tiny one-shot, emitted last so the ACT table
            # load doesn't sit in front of Scalar's DMA issues.
            std_sb = wp.tile([1, O], mybir.dt.float32)
            nc.scalar.activation(std_sb[:], ls_sb[:], mybir.ActivationFunctionType.Exp)
            nc.gpsimd.dma_start(out=std_d[:], in_=std_sb[:])

    nc.compile()
    return nc


def kernel(x, index, W, b, logstd, *, trace=False, mode=None):
    global LAST_RESULTS
    mode = mode or DEFAULT_MODE
    x = np.ascontiguousarray(x, dtype=np.float32)
    W = np.ascontiguousarray(W, dtype=np.float32)
    b = np.ascontiguousarray(b, dtype=np.float32)
    logstd = np.ascontiguousarray(logstd, dtype=np.float32)
    idx = np.asarray(index).astype(np.int64)

    order = np.argsort(idx, kind="stable")
    counts = np.bincount(idx, minlength=E)
    starts = np.zeros(E + 1, dtype=np.int64)
    np.cumsum(counts, out=starts[1:])

    C = max(C_DEFAULT, math.ceil(counts.max() / 128) * 128)
    key = (C, mode)
    nc = _PROGRAM_CACHE.get(key)
    if nc is None:
        build = _build_program_raw if mode.endswith("-raw") else _build_program
        nc = _PROGRAM_CACHE[key] = build(C, mode)

    np_dt = ml_dtypes.bfloat16 if mode.startswith("bf16") else np.float32
    in_maps = []
    for e in range(E):
        rows = order[starts[e]:starts[e + 1]]
        xgT = np.zeros((I, C), dtype=np_dt)
        xgT[:, :len(rows)] = x[rows].astype(np_dt).T
        in_maps.append({
            "xT": xgT,
            "w": W[e].astype(np_dt),
            "bias": np.ascontiguousarray(np.broadcast_to(b[e], (128, O))),
            "ls": logstd.reshape(1, O),
        })

    res = run_bass_kernel_spmd(nc, in_maps, list(range(NCORES)), trace=trace)
    LAST_RESULTS = res

    action_mean = np.empty((B, O), dtype=np.float32)
    for e in range(E):
        n = counts[e]
        action_mean[order[starts[e]:starts[e + 1]]] = res.results[e]["y"][:n]
    std_row = res.results[0]["std"][0]
    action_std = np.ascontiguousarray(np.broadcast_to(std_row, (B, O)))
    return action_mean, action_std
